# revision 1
# baseline (speedup 1.0000x reference)
"""EdgeOnlyConv GNN message-passing kernel for Trainium2 (8 NeuronCores).

out[e] = concat(x[src[e]], x[dest[e]], edge_attr[e]) @ W.T + b

Strategy (edge-parallel across 8 cores, x & weights replicated):
  Phase A (per core): node tables Ys = x @ W_src.T + b, Yd = x @ W_dest.T,
    stored fp16 as PAIR-ROW tables [N/2, 256] (row k = nodes 2k,2k+1).
  Phase B (per core), per 2048-edge supertile:
    - one dma_gather per endpoint table (int16 pair indices = node>>1,
      2048 idx/call) fetches both nodes of each pair (512B rows)
    - DVE parity select picks the right half per edge (host parity masks)
    - z = edge_attr @ W_edge.T on PE (edge_attr passed host-transposed)
    - out = sel_src + sel_dst + z, batched store
"""

import sys
import numpy as np

if "/opt/trn_rl_repo" not in sys.path:
    sys.path.insert(0, "/opt/trn_rl_repo")

P = 128
CHUNK_IDX = 1024   # indices per dma_gather call (HW descriptor-ring limit)

N_CORES = 8
N_NODES = 50000
N_IN_NODE = 128
N_IN_EDGE = 64
N_OUT = 128
N_EDGES = 1000000
E_CORE = N_EDGES // N_CORES          # 125000
K_SUP = 16                           # 128-edge tiles per supertile
T_TILES = ((E_CORE + P - 1) // P + K_SUP - 1) // K_SUP * K_SUP   # 992
E_PAD = T_TILES * P                  # 126976
S_SUP = T_TILES // K_SUP             # 62
NODES_PAD = (N_NODES + 255) // 256 * 256   # 50176 (pair rows: 25088)
A_TILES = NODES_PAD // P             # 392


def build_program(
    n_cores=N_CORES,
    nodes_pad=NODES_PAD,
    e_pad=E_PAD,
    k_sup=K_SUP,
):
    """Build the Bass program. Returns the compiled Bacc object."""
    import concourse.mybir as mybir
    import concourse.tile as tile
    from concourse import bacc
    from concourse import bass as cbass

    f32 = mybir.dt.float32
    f16 = mybir.dt.float16
    i16 = mybir.dt.int16

    a_tiles = nodes_pad // P
    t_tiles = e_pad // P
    s_sup = t_tiles // k_sup
    n_idx = k_sup * P                       # indices per dma_gather call
    idx_cols = n_idx // 16                  # int16 idx columns per supertile
    d_comb = 2 * N_OUT                      # 256
    pair_rows = nodes_pad // 2

    nc = bacc.Bacc("TRN2", target_bir_lowering=False, debug=False,
                   num_devices=n_cores)

    x_d = nc.dram_tensor("x", [nodes_pad, N_IN_NODE], f16, kind="ExternalInput").ap()
    wct_d = nc.dram_tensor("wct", [N_IN_NODE, d_comb], f16, kind="ExternalInput").ap()
    wet_d = nc.dram_tensor("wet", [N_IN_EDGE, N_OUT], f32, kind="ExternalInput").ap()
    bias_d = nc.dram_tensor("bias", [P, 2 * d_comb], f32, kind="ExternalInput").ap()
    gs_d = nc.dram_tensor("gs", [P, s_sup * idx_cols], i16, kind="ExternalInput").ap()
    gd_d = nc.dram_tensor("gd", [P, s_sup * idx_cols], i16, kind="ExternalInput").ap()
    ps_d = nc.dram_tensor("ps", [P, 2 * t_tiles], f16, kind="ExternalInput").ap()
    pd_d = nc.dram_tensor("pd", [P, 2 * t_tiles], f16, kind="ExternalInput").ap()
    eat_d = nc.dram_tensor("eat", [N_IN_EDGE, e_pad], f32, kind="ExternalInput").ap()
    out_d = nc.dram_tensor("out", [e_pad, N_OUT], f32, kind="ExternalOutput").ap()
    ys_d = nc.dram_tensor("ys", [pair_rows, d_comb], f16, kind="Internal").ap()
    yd_d = nc.dram_tensor("yd", [pair_rows, d_comb], f16, kind="Internal").ap()
    # node-row views of the pair tables for phase A stores
    ys_v = ys_d.rearrange("k (j f) -> (k j) f", j=2)
    yd_v = yd_d.rearrange("k (j f) -> (k j) f", j=2)

    GRP = 8  # node tiles per phase-A group

    with tile.TileContext(nc) as tc:
        with tc.tile_pool(name="static", bufs=1) as spool:
            wct_sb = spool.tile([N_IN_NODE, d_comb], f16)
            nc.sync.dma_start(wct_sb[:], wct_d[:, :])
            wet_sb = spool.tile([N_IN_EDGE, N_OUT], f32)
            nc.sync.dma_start(wet_sb[:], wet_d[:, :])
            bias_sb = spool.tile([P, 2 * d_comb], f32)
            nc.sync.dma_start(bias_sb[:], bias_d[:, :])
            gs_sb = spool.tile([P, s_sup * idx_cols], i16)
            nc.sync.dma_start(gs_sb[:], gs_d[:, :])
            gd_sb = spool.tile([P, s_sup * idx_cols], i16)
            nc.sync.dma_start(gd_sb[:], gd_d[:, :])
            ps_sb = spool.tile([P, 2 * t_tiles], f16)
            nc.sync.dma_start(ps_sb[:], ps_d[:, :])
            pd_sb = spool.tile([P, 2 * t_tiles], f16)
            nc.sync.dma_start(pd_sb[:], pd_d[:, :])

            # ---- Phase A: Ys = x @ Wsrc.T + b, Yd = x @ Wdest.T (fp16) ----
            with tc.tile_pool(name="a_sbuf", bufs=3) as apool, \
                 tc.tile_pool(name="a_ps_yc", bufs=4, space="PSUM") as aps_yc:
                for g0 in range(0, a_tiles, GRP):
                    gn = min(GRP, a_tiles - g0)
                    xt_sb = apool.tile([P, GRP * P], f16, tag="xt_sb")
                    nc.sync.dma_start(
                        xt_sb[:, :gn * P],
                        x_d[g0 * P:(g0 + gn) * P, :], transpose=True)
                    yc_sb = apool.tile([P, GRP * d_comb], f16, tag="yc_sb")
                    for h0 in range(0, gn, 2):
                        hn = min(2, gn - h0)
                        yc_ps = aps_yc.tile([P, 2 * d_comb], f32, tag="yc_ps")
                        for i in range(h0, h0 + hn):
                            nc.tensor.matmul(
                                yc_ps[:, (i - h0) * d_comb:(i - h0 + 1) * d_comb],
                                lhsT=xt_sb[:, i * P:(i + 1) * P],
                                rhs=wct_sb[:], start=True, stop=True)
                        nc.vector.tensor_add(
                            yc_sb[:, h0 * d_comb:(h0 + hn) * d_comb],
                            yc_ps[:, :hn * d_comb],
                            bias_sb[:, :hn * d_comb])
                    # batched stores: ys rows g0*P..(g0+gn)*P from strided cols
                    yc_v = yc_sb.rearrange("p (g c) -> p g c", c=d_comb)
                    ys_rows = ys_v[g0 * P:(g0 + gn) * P, :].rearrange(
                        "(g p) f -> p g f", p=P)
                    yd_rows = yd_v[g0 * P:(g0 + gn) * P, :].rearrange(
                        "(g p) f -> p g f", p=P)
                    nc.sync.dma_start(ys_rows[:, :, :], yc_v[:, :gn, 0:N_OUT])
                    nc.sync.dma_start(yd_rows[:, :, :], yc_v[:, :gn, N_OUT:d_comb])

            tc.strict_bb_all_engine_barrier()

            # ---- Phase B ----
            out_v = out_d.rearrange("(t p) o -> p t o", p=P)
            sup_cols = k_sup * P
            with tc.tile_pool(name="b_sbuf", bufs=2) as bpool, \
                 tc.tile_pool(name="b_psum", bufs=4, space="PSUM") as bpsum:
                for s in range(s_sup):
                    j0 = s * k_sup
                    # 512-idx chunks: larger single dma_gather calls overflow
                    # the SWDGE descriptor ring and hang the device
                    ch_idx = min(CHUNK_IDX, n_idx)
                    ch_tiles = ch_idx // P
                    ch_cols = ch_idx // 16
                    n_ch = n_idx // ch_idx
                    gsrc = bpool.tile([P, k_sup, d_comb], f16, tag="gsrc")
                    gdst = bpool.tile([P, k_sup, d_comb], f16, tag="gdst")
                    for c in range(n_ch):
                        c0 = s * idx_cols + c * ch_cols
                        nc.gpsimd.dma_gather(
                            out_ap=gsrc[:, c * ch_tiles:(c + 1) * ch_tiles, :],
                            in_ap=ys_d[:, :],
                            idxs_ap=gs_sb[:, c0:c0 + ch_cols],
                            num_idxs=ch_idx, num_idxs_reg=ch_idx,
                            elem_size=d_comb)
                        nc.gpsimd.dma_gather(
                            out_ap=gdst[:, c * ch_tiles:(c + 1) * ch_tiles, :],
                            in_ap=yd_d[:, :],
                            idxs_ap=gd_sb[:, c0:c0 + ch_cols],
                            num_idxs=ch_idx, num_idxs_reg=ch_idx,
                            elem_size=d_comb)
                    eat_sb = bpool.tile([N_IN_EDGE, sup_cols], f32, tag="eat_sb")
                    nc.sync.dma_start(
                        eat_sb[:], eat_d[:, j0 * P:(j0 + k_sup) * P])

                    # parity select: res = lo + par*(hi-lo), per endpoint
                    par_s = ps_sb[:, 2 * j0:2 * (j0 + k_sup)].rearrange(
                        "p (g two) -> p g two", two=2)
                    par_d = pd_sb[:, 2 * j0:2 * (j0 + k_sup)].rearrange(
                        "p (g two) -> p g two", two=2)
                    us = bpool.tile([P, k_sup, N_OUT], f16, tag="us")
                    nc.vector.tensor_sub(
                        us[:, :, :], gsrc[:, :, N_OUT:d_comb], gsrc[:, :, 0:N_OUT])
                    nc.vector.tensor_mul(
                        us[:, :, :], us[:, :, :],
                        par_s[:, :, 0:1].to_broadcast([P, k_sup, N_OUT]))
                    ud = bpool.tile([P, k_sup, N_OUT], f16, tag="ud")
                    nc.vector.tensor_sub(
                        ud[:, :, :], gdst[:, :, N_OUT:d_comb], gdst[:, :, 0:N_OUT])
                    nc.vector.tensor_mul(
                        ud[:, :, :], ud[:, :, :],
                        par_d[:, :, 0:1].to_broadcast([P, k_sup, N_OUT]))
                    q = bpool.tile([P, k_sup, N_OUT], f32, tag="q")
                    nc.vector.tensor_add(
                        q[:, :, :], gsrc[:, :, 0:N_OUT], gdst[:, :, 0:N_OUT])
                    tsum = bpool.tile([P, k_sup, N_OUT], f32, tag="tsum")
                    nc.vector.tensor_add(tsum[:, :, :], us[:, :, :], ud[:, :, :])
                    nc.vector.tensor_add(tsum[:, :, :], tsum[:, :, :], q[:, :, :])

                    outsb = bpool.tile([P, sup_cols], f32, tag="outsb")
                    tsum_f = tsum.rearrange("p g o -> p (g o)")
                    for bank in range(k_sup // 4):
                        z_ps = bpsum.tile([P, 4 * P], f32, tag="z_ps")
                        for jj in range(4):
                            t_loc = bank * 4 + jj
                            nc.tensor.matmul(
                                z_ps[:, jj * P:(jj + 1) * P],
                                lhsT=eat_sb[:, t_loc * P:(t_loc + 1) * P],
                                rhs=wet_sb[:], start=True, stop=True)
                        nc.vector.tensor_add(
                            outsb[:, bank * 4 * P:(bank + 1) * 4 * P],
                            z_ps[:], tsum_f[:, bank * 4 * P:(bank + 1) * 4 * P])
                    nc.sync.dma_start(out_v[:, j0:j0 + k_sup, :], outsb[:])

    nc.compile()
    return nc


def _idx_wrap16(seq_i16, n_idx):
    """Pack a flat int16 index sequence into the dma_gather SBUF layout:
    index i at (partition i%16, column i//16), replicated to 8x16 rows."""
    cols = n_idx // 16
    blocks = seq_i16.reshape(-1, cols, 16)           # [S, cols, 16]
    arr = blocks.transpose(0, 2, 1).reshape(-1, 16, cols)  # [S, 16, cols]
    out = np.concatenate([np.tile(a, (8, 1)) for a in arr], axis=1)
    return np.ascontiguousarray(out)                 # [128, S*cols]


def prep_inputs(x, edge_index, edge_attr, W, b,
                n_cores=N_CORES, e_pad=E_PAD, nodes_pad=NODES_PAD,
                k_sup=K_SUP):
    """Host-side input prep: shard + pad + layout. Returns list of in_maps."""
    x = np.asarray(x, dtype=np.float32)
    edge_index = np.asarray(edge_index)
    edge_attr = np.asarray(edge_attr, dtype=np.float32)
    W = np.asarray(W, dtype=np.float32)
    b = np.asarray(b, dtype=np.float32)

    n_nodes, d_node = x.shape
    e_total = edge_index.shape[1]
    e_core = e_total // n_cores
    d_out = W.shape[0]
    d_edge = edge_attr.shape[1]
    t_tiles = e_pad // P
    n_idx = k_sup * P

    x_pad = np.zeros((nodes_pad, d_node), dtype=np.float16)
    x_pad[:n_nodes] = x.astype(np.float16)
    wct = np.ascontiguousarray(np.concatenate(
        [W[:, :d_node].T, W[:, d_node:2 * d_node].T], axis=1)).astype(np.float16)
    wet = np.ascontiguousarray(W[:, 2 * d_node:].T)
    bias_comb = np.concatenate(
        [np.tile(b, (P, 1)), np.zeros((P, d_out), dtype=np.float32)], axis=1)
    bias_full = np.ascontiguousarray(
        np.tile(bias_comb, (1, 2)).astype(np.float32))

    src = np.ascontiguousarray(edge_index[0]).astype(np.int32)
    dst = np.ascontiguousarray(edge_index[1]).astype(np.int32)

    in_maps = []
    for c in range(n_cores):
        lo, hi = c * e_core, (c + 1) * e_core
        src_pad = np.zeros(e_pad, dtype=np.int32)
        src_pad[:e_core] = src[lo:hi]
        dst_pad = np.zeros(e_pad, dtype=np.int32)
        dst_pad[:e_core] = dst[lo:hi]
        chunk = min(CHUNK_IDX, n_idx)
        gs = _idx_wrap16((src_pad >> 1).astype(np.int16), chunk)
        gd = _idx_wrap16((dst_pad >> 1).astype(np.int16), chunk)
        # parity masks in t-major tile layout, duplicated (mask, 0) pairs so
        # device can broadcast-slice [:, :, 0:1]
        ps = np.zeros((P, 2 * t_tiles), dtype=np.float16)
        ps[:, 0::2] = (src_pad & 1).astype(np.float16).reshape(t_tiles, P).T
        pd = np.zeros((P, 2 * t_tiles), dtype=np.float16)
        pd[:, 0::2] = (dst_pad & 1).astype(np.float16).reshape(t_tiles, P).T
        ea_pad = np.zeros((e_pad, d_edge), dtype=np.float32)
        ea_pad[:e_core] = edge_attr[lo:hi]
        eat = np.ascontiguousarray(ea_pad.T)
        in_maps.append({
            "x": x_pad, "wct": wct, "wet": wet, "bias": bias_full,
            "gs": gs, "gd": gd, "ps": ps, "pd": pd, "eat": eat,
        })
    return in_maps


_NC_CACHE = {}


def _get_program():
    key = "full"
    if key not in _NC_CACHE:
        _NC_CACHE[key] = build_program()
    return _NC_CACHE[key]


def run_on_hw(in_maps, nc=None, trace=False, n_cores=N_CORES):
    from concourse import bass_utils
    if nc is None:
        nc = _get_program()
    kw = {}
    if trace:
        _install_profile_hook(bass_utils)
        kw["trace"] = True
    res = bass_utils.run_bass_kernel_spmd(
        nc, in_maps, core_ids=list(range(n_cores)), **kw)
    return res


def _install_profile_hook(bass_utils):
    """Inject the NTFF profile hook missing from this image's antenv."""
    import types
    if "antenv.axon_hooks" in sys.modules:
        return
    try:
        from trn_agent_boot.trn_boot import _ntff_profile_via_ctypes
        hook = _ntff_profile_via_ctypes("/opt/axon/libaxon_pjrt.so")
    except Exception:
        hook = None
    mod = types.ModuleType("antenv.axon_hooks")
    mod.get_axon_ntff_profile_hook = lambda: hook
    mod.set_axon_ntff_profile_hook = lambda h: None
    sys.modules["antenv.axon_hooks"] = mod
    bass_utils.upload_artifacts = lambda tmpdir: f"file://{tmpdir}"


def kernel(x, edge_index, edge_attr, W, b):
    in_maps = prep_inputs(x, edge_index, edge_attr, W, b)
    res = run_on_hw(in_maps)
    e_core = edge_index.shape[1] // N_CORES
    outs = [res.results[c]["out"][:e_core] for c in range(N_CORES)]
    return np.concatenate(outs, axis=0)



# revision 4
# speedup vs baseline: 2.5958x; 2.5958x over previous
"""EdgeOnlyConv GNN message-passing kernel for Trainium2 (8 NeuronCores).

out[e] = concat(x[src[e]], x[dest[e]], edge_attr[e]) @ W.T + b

Strategy (edge-parallel across 8 cores, x & weights replicated):
  Phase A (per core): combined node table C[k] =
    [Ys[2k] | Yd[2k] | Ys[2k+1] | Yd[2k+1]]  (f16, 25088 pair rows x 1KB)
    where Ys = x @ Wsrc.T + b, Yd = x @ Wdst.T. Stored with 512B descriptors.
  Phase B (per core): edges host-sorted into 4 parity groups (src&1, dst&1)
    of 32768 slots each. Per 2048-edge supertile (one parity group):
    - 4 dma_gather calls (2 per endpoint, 1024 int16 pair indices each,
      256B reads via elem_step=512 at the group's static parity offset),
      rotated over 4 SWDGE queues so descriptor-gen and DMA overlap.
    - z = edge_attr @ We.T on PE (f16, host-transposed edge_attr)
    - DVE: q = gsrc + gdst;  out = psum(z) + q  (fused scalar_tensor_tensor)
    - f16 output, big-descriptor store; host undoes the permutation.
"""

import sys
import numpy as np

if "/opt/trn_rl_repo" not in sys.path:
    sys.path.insert(0, "/opt/trn_rl_repo")

P = 128
CALL_IDX = 1024          # indices per dma_gather call (SWDGE ring limit)

N_CORES = 8
N_NODES = 50000
N_IN_NODE = 128
N_IN_EDGE = 64
N_OUT = 128
N_EDGES = 1000000
E_CORE = N_EDGES // N_CORES          # 125000
GROUP_SLOTS = 32768                  # slots per parity group (32 calls)
N_GROUPS = 4
E_PAD = N_GROUPS * GROUP_SLOTS       # 131072
K_SUP = 16                           # 128-edge tiles per supertile
S_SUP = E_PAD // (K_SUP * P)         # 64 supertiles
NODES_PAD = (N_NODES + 255) // 256 * 256   # 50176
A_TILES = NODES_PAD // P             # 392
PAIR_ROWS = NODES_PAD // 2           # 25088


def build_program():
    import concourse.mybir as mybir
    import concourse.tile as tile
    from concourse import bacc

    f16 = mybir.dt.float16
    f32 = mybir.dt.float32
    i16 = mybir.dt.int16
    Copy = mybir.ActivationFunctionType.Copy
    Alu = mybir.AluOpType

    d_comb = 2 * N_OUT                      # 256
    idx_cols = CALL_IDX // 16               # 64 int16 cols per call
    sup_cols = K_SUP * P                    # 2048
    calls_per_side = sup_cols // CALL_IDX   # 2

    nc = bacc.Bacc("TRN2", target_bir_lowering=False, debug=False,
                   num_devices=N_CORES, num_swdge_queues=4)

    xt_d = nc.dram_tensor("xt", [P, NODES_PAD], f16, kind="ExternalInput").ap()
    wct_d = nc.dram_tensor("wct", [N_IN_NODE, d_comb], f16, kind="ExternalInput").ap()
    wet_d = nc.dram_tensor("wet", [N_IN_EDGE, N_OUT], f16, kind="ExternalInput").ap()
    ones_d = nc.dram_tensor("ones", [1, P], f16, kind="ExternalInput").ap()
    brow_d = nc.dram_tensor("brow", [1, d_comb], f16, kind="ExternalInput").ap()
    gs_d = nc.dram_tensor("gs", [P, E_PAD // 16], i16, kind="ExternalInput").ap()
    gd_d = nc.dram_tensor("gd", [P, E_PAD // 16], i16, kind="ExternalInput").ap()
    eat_d = nc.dram_tensor("eat", [N_IN_EDGE, E_PAD], f16, kind="ExternalInput").ap()
    out_d = nc.dram_tensor("out", [P, E_PAD], f16, kind="ExternalOutput").ap()
    c_d = nc.dram_tensor("ctab", [PAIR_ROWS, 2 * d_comb], f16, kind="Internal").ap()
    # node-row view for phase A stores: row n = [Ys[n](128) | Yd[n](128)]
    c_nodes = c_d.rearrange("k (j f) -> (k j) f", j=2)

    GRP = 8  # node tiles per phase-A group

    with tile.TileContext(nc) as tc:
        with tc.tile_pool(name="static", bufs=1) as spool:
            wct_sb = spool.tile([N_IN_NODE, d_comb], f16)
            nc.sync.dma_start(wct_sb[:], wct_d[:, :])
            wet_sb = spool.tile([N_IN_EDGE, N_OUT], f16)
            nc.sync.dma_start(wet_sb[:], wet_d[:, :])
            ones_sb = spool.tile([1, P], f16)
            nc.sync.dma_start(ones_sb[:], ones_d[:, :])
            brow_sb = spool.tile([1, d_comb], f16)
            nc.sync.dma_start(brow_sb[:], brow_d[:, :])
            gs_sb = spool.tile([P, E_PAD // 16], i16)
            nc.sync.dma_start(gs_sb[:], gs_d[:, :])
            gd_sb = spool.tile([P, E_PAD // 16], i16)
            nc.sync.dma_start(gd_sb[:], gd_d[:, :])

            # ---- Phase A: C pair table from xT (feature-major) ----
            with tc.tile_pool(name="a_sbuf", bufs=2) as apool, \
                 tc.tile_pool(name="a_x", bufs=1) as xpool, \
                 tc.tile_pool(name="a_ps", bufs=8, space="PSUM") as apsum:
                xt_sb = xpool.tile([P, NODES_PAD], f16)
                half = NODES_PAD // 2
                nc.sync.dma_start(xt_sb[:, :half], xt_d[:, :half])
                nc.sync.dma_start(xt_sb[:, half:], xt_d[:, half:])
                for g0 in range(0, A_TILES, GRP):
                    yc_sb = apool.tile([P, GRP, d_comb], f16, tag="yc_sb")
                    for i in range(g0, g0 + GRP):
                        yc_ps = apsum.tile([P, d_comb], f32, tag="yc_ps")
                        nc.tensor.matmul(
                            yc_ps[:, :],
                            lhsT=xt_sb[:, i * P:(i + 1) * P],
                            rhs=wct_sb[:], start=True, stop=False)
                        nc.tensor.matmul(
                            yc_ps[:, :], lhsT=ones_sb[:, :], rhs=brow_sb[:, :],
                            start=False, stop=True)
                        nc.scalar.activation(
                            yc_sb[:, i - g0, :], yc_ps[:, :], Copy)
                    rows = c_nodes[g0 * P:(g0 + GRP) * P, :].rearrange(
                        "(g p) f -> p g f", p=P)
                    nc.sync.dma_start(rows[:, :, :], yc_sb[:, :, :])

            tc.strict_bb_all_engine_barrier()

            # ---- Phase B ----
            with tc.tile_pool(name="b_sbuf", bufs=3) as bpool, \
                 tc.tile_pool(name="b_ps", bufs=8, space="PSUM") as bpsum:
                for s in range(S_SUP):
                    grp = s // (S_SUP // N_GROUPS)
                    sp, dp = (grp >> 1) & 1, grp & 1
                    src_off = sp * 2 * N_OUT            # 0 or 512 (elems)
                    dst_off = N_OUT + dp * 2 * N_OUT    # 128 or 640... see map
                    gsrc = bpool.tile([P, K_SUP, N_OUT], f16, tag="gsrc")
                    gdst = bpool.tile([P, K_SUP, N_OUT], f16, tag="gdst")
                    for c in range(calls_per_side):
                        c0 = (s * sup_cols // 16) + c * idx_cols
                        q_base = (s * 2 * calls_per_side + 2 * c) % 4
                        nc.gpsimd.dma_gather(
                            out_ap=gsrc[:, c * 8:(c + 1) * 8, :],
                            in_ap=c_d[:, src_off:src_off + N_OUT],
                            idxs_ap=gs_sb[:, c0:c0 + idx_cols],
                            num_idxs=CALL_IDX, num_idxs_reg=CALL_IDX,
                            elem_size=N_OUT, elem_step=2 * d_comb,
                            queue_num=q_base)
                        nc.gpsimd.dma_gather(
                            out_ap=gdst[:, c * 8:(c + 1) * 8, :],
                            in_ap=c_d[:, dst_off:dst_off + N_OUT],
                            idxs_ap=gd_sb[:, c0:c0 + idx_cols],
                            num_idxs=CALL_IDX, num_idxs_reg=CALL_IDX,
                            elem_size=N_OUT, elem_step=2 * d_comb,
                            queue_num=q_base + 1)
                    eat_sb = bpool.tile([N_IN_EDGE, K_SUP, P], f16, tag="eat_sb")
                    nc.sync.dma_start(
                        eat_sb[:, :, :],
                        eat_d[:, s * sup_cols:(s + 1) * sup_cols].rearrange(
                            "f (t p) -> f t p", p=P))
                    q = bpool.tile([P, K_SUP, N_OUT], f16, tag="q")
                    nc.vector.tensor_add(q[:, :, :], gsrc[:, :, :], gdst[:, :, :])
                    outsb = bpool.tile([P, K_SUP, N_OUT], f16, tag="outsb")
                    for b in range(K_SUP // 4):
                        z_ps = bpsum.tile([P, 4 * P], f32, tag="z_ps")
                        for j in range(4):
                            nc.tensor.matmul(
                                z_ps[:, j * P:(j + 1) * P],
                                lhsT=eat_sb[:, b * 4 + j, :],
                                rhs=wet_sb[:], start=True, stop=True)
                        nc.vector.scalar_tensor_tensor(
                            outsb[:, b * 4:(b + 1) * 4, :],
                            in0=z_ps[:, :], scalar=1.0,
                            in1=q[:, b * 4:(b + 1) * 4, :],
                            op0=Alu.bypass, op1=Alu.add)
                    nc.sync.dma_start(
                        out_d[:, s * sup_cols:(s + 1) * sup_cols],
                        outsb.rearrange("p t o -> p (t o)")[:, :])

    nc.compile()
    return nc


def prep_inputs(x, edge_index, edge_attr, W, b):
    """Host-side prep: parity-group sort per core, f16 casts, layouts."""
    x = np.asarray(x, dtype=np.float32)
    edge_index = np.asarray(edge_index)
    edge_attr = np.asarray(edge_attr, dtype=np.float32)
    W = np.asarray(W, dtype=np.float32)
    b = np.asarray(b, dtype=np.float32)

    d_node = x.shape[1]
    xt = np.zeros((P, NODES_PAD), dtype=np.float16)
    xt[:, :x.shape[0]] = x.T.astype(np.float16)
    wct = np.ascontiguousarray(np.concatenate(
        [W[:, :d_node].T, W[:, d_node:2 * d_node].T], axis=1)).astype(np.float16)
    wet = np.ascontiguousarray(W[:, 2 * d_node:].T).astype(np.float16)
    ones = np.ones((1, P), dtype=np.float16)
    brow = np.zeros((1, 2 * N_OUT), dtype=np.float16)
    brow[0, :N_OUT] = b.astype(np.float16)

    src = np.ascontiguousarray(edge_index[0]).astype(np.int32)
    dst = np.ascontiguousarray(edge_index[1]).astype(np.int32)

    in_maps = []
    perms = []
    for c in range(N_CORES):
        lo, hi = c * E_CORE, (c + 1) * E_CORE
        sc, dc = src[lo:hi], dst[lo:hi]
        grp = (sc & 1) * 2 + (dc & 1)
        slot_to_edge = np.full(E_PAD, -1, dtype=np.int32)
        gs = np.zeros(E_PAD, dtype=np.int16)
        gd = np.zeros(E_PAD, dtype=np.int16)
        for g in range(N_GROUPS):
            idx_e = np.nonzero(grp == g)[0]
            n = idx_e.size
            assert n <= GROUP_SLOTS, f"parity group overflow: {n}"
            base = g * GROUP_SLOTS
            slot_to_edge[base:base + n] = idx_e
            gs[base:base + n] = (sc[idx_e] >> 1).astype(np.int16)
            gd[base:base + n] = (dc[idx_e] >> 1).astype(np.int16)
        valid = slot_to_edge >= 0
        ea_slot = np.zeros((E_PAD, N_IN_EDGE), dtype=np.float16)
        ea_slot[valid] = edge_attr[lo + slot_to_edge[valid]].astype(np.float16)
        eat = np.ascontiguousarray(ea_slot.T)
        in_maps.append({
            "xt": xt, "wct": wct, "wet": wet, "ones": ones, "brow": brow,
            "gs": _idx_wrap16(gs, CALL_IDX), "gd": _idx_wrap16(gd, CALL_IDX),
            "eat": eat,
        })
        perms.append(slot_to_edge)
    return in_maps, perms


def _idx_wrap16(seq_i16, n_idx):
    """Pack a flat int16 index sequence into the dma_gather SBUF layout:
    index i of each n_idx-call at (partition i%16, column i//16), replicated
    to 8x16 partition rows."""
    cols = n_idx // 16
    blocks = seq_i16.reshape(-1, cols, 16)
    arr = blocks.transpose(0, 2, 1).reshape(-1, 16, cols)
    out = np.concatenate([np.tile(a, (8, 1)) for a in arr], axis=1)
    return np.ascontiguousarray(out)


def unpack_outputs(res, perms):
    outs = []
    for c in range(N_CORES):
        o = res.results[c]["out"]                       # [128, E_PAD] f16
        # cols are (s, t, chan); slot = s*2048 + t*128 + p
        rows = np.ascontiguousarray(
            o.reshape(P, S_SUP, K_SUP, N_OUT).transpose(1, 2, 0, 3)
            .reshape(E_PAD, N_OUT))
        slot_to_edge = perms[c]
        valid = slot_to_edge >= 0
        oc = np.empty((E_CORE, N_OUT), dtype=np.float32)
        oc[slot_to_edge[valid]] = rows[valid].astype(np.float32)
        outs.append(oc)
    return np.concatenate(outs, axis=0)


_NC_CACHE = {}


def _get_program():
    key = "full"
    if key not in _NC_CACHE:
        _NC_CACHE[key] = build_program()
    return _NC_CACHE[key]


def run_on_hw(in_maps, nc=None, trace=False):
    from concourse import bass_utils
    if nc is None:
        nc = _get_program()
    kw = {}
    if trace:
        _install_profile_hook(bass_utils)
        kw["trace"] = True
    res = bass_utils.run_bass_kernel_spmd(
        nc, in_maps, core_ids=list(range(N_CORES)), **kw)
    return res


def _install_profile_hook(bass_utils):
    """Inject the NTFF profile hook missing from this image's antenv."""
    import types
    if "antenv.axon_hooks" in sys.modules:
        return
    try:
        from trn_agent_boot.trn_boot import _ntff_profile_via_ctypes
        hook = _ntff_profile_via_ctypes("/opt/axon/libaxon_pjrt.so")
    except Exception:
        hook = None
    mod = types.ModuleType("antenv.axon_hooks")
    mod.get_axon_ntff_profile_hook = lambda: hook
    mod.set_axon_ntff_profile_hook = lambda h: None
    sys.modules["antenv.axon_hooks"] = mod
    bass_utils.upload_artifacts = lambda tmpdir: f"file://{tmpdir}"


def kernel(x, edge_index, edge_attr, W, b):
    in_maps, perms = prep_inputs(x, edge_index, edge_attr, W, b)
    res = run_on_hw(in_maps)
    return unpack_outputs(res, perms)


# revision 5
# speedup vs baseline: 2.6166x; 1.0080x over previous
"""EdgeOnlyConv GNN message-passing kernel for Trainium2 (8 NeuronCores).

out[e] = concat(x[src[e]], x[dest[e]], edge_attr[e]) @ W.T + b

Strategy (edge-parallel across 8 cores, x & weights replicated):
  Phase A (per core): combined node table C[k] =
    [Ys[2k] | Yd[2k] | Ys[2k+1] | Yd[2k+1]]  (f16, 25088 pair rows x 1KB)
    where Ys = x @ Wsrc.T + b, Yd = x @ Wdst.T. Stored with 512B descriptors.
  Phase B (per core): edges host-sorted into 4 parity groups (src&1, dst&1)
    of 32768 slots each. Per 2048-edge supertile (one parity group):
    - 4 dma_gather calls (2 per endpoint, 1024 int16 pair indices each,
      256B reads via elem_step=512 at the group's static parity offset),
      rotated over 4 SWDGE queues so descriptor-gen and DMA overlap.
    - z = edge_attr @ We.T on PE (f16, host-transposed edge_attr)
    - DVE: q = gsrc + gdst;  out = psum(z) + q  (fused scalar_tensor_tensor)
    - f16 output, big-descriptor store; host undoes the permutation.
"""

import sys
import numpy as np

if "/opt/trn_rl_repo" not in sys.path:
    sys.path.insert(0, "/opt/trn_rl_repo")

P = 128
CALL_IDX = 1024          # indices per dma_gather call (SWDGE ring limit)

N_CORES = 8
N_NODES = 50000
N_IN_NODE = 128
N_IN_EDGE = 64
N_OUT = 128
N_EDGES = 1000000
E_CORE = N_EDGES // N_CORES          # 125000
GROUP_SLOTS = 32768                  # slots per parity group (32 calls)
N_GROUPS = 4
E_PAD = N_GROUPS * GROUP_SLOTS       # 131072
K_SUP = 16                           # 128-edge tiles per supertile
S_SUP = E_PAD // (K_SUP * P)         # 64 supertiles
NODES_PAD = (N_NODES + 255) // 256 * 256   # 50176
A_TILES = NODES_PAD // P             # 392
PAIR_ROWS = NODES_PAD // 2           # 25088


def build_program():
    import concourse.mybir as mybir
    import concourse.tile as tile
    from concourse import bacc

    f16 = mybir.dt.float16
    f32 = mybir.dt.float32
    i16 = mybir.dt.int16
    Copy = mybir.ActivationFunctionType.Copy
    Alu = mybir.AluOpType

    d_comb = 2 * N_OUT                      # 256
    idx_cols = CALL_IDX // 16               # 64 int16 cols per call
    sup_cols = K_SUP * P                    # 2048
    calls_per_side = sup_cols // CALL_IDX   # 2

    nc = bacc.Bacc("TRN2", target_bir_lowering=False, debug=False,
                   num_devices=N_CORES, num_swdge_queues=4)

    xt_d = nc.dram_tensor("xt", [P, NODES_PAD], f16, kind="ExternalInput").ap()
    wct_d = nc.dram_tensor("wct", [N_IN_NODE, d_comb], f16, kind="ExternalInput").ap()
    wet_d = nc.dram_tensor("wet", [N_IN_EDGE, N_OUT], f16, kind="ExternalInput").ap()
    ones_d = nc.dram_tensor("ones", [1, P], f16, kind="ExternalInput").ap()
    brow_d = nc.dram_tensor("brow", [1, d_comb], f16, kind="ExternalInput").ap()
    gs_d = nc.dram_tensor("gs", [P, E_PAD // 16], i16, kind="ExternalInput").ap()
    gd_d = nc.dram_tensor("gd", [P, E_PAD // 16], i16, kind="ExternalInput").ap()
    eat_d = nc.dram_tensor("eat", [N_IN_EDGE, E_PAD], f16, kind="ExternalInput").ap()
    out_d = nc.dram_tensor("out", [P, E_PAD], f16, kind="ExternalOutput").ap()
    c_d = nc.dram_tensor("ctab", [PAIR_ROWS, 2 * d_comb], f16, kind="Internal").ap()
    # node-row view for phase A stores: row n = [Ys[n](128) | Yd[n](128)]
    c_nodes = c_d.rearrange("k (j f) -> (k j) f", j=2)

    GRP = 8  # node tiles per phase-A group

    with tile.TileContext(nc) as tc:
        with tc.tile_pool(name="static", bufs=1) as spool:
            wct_sb = spool.tile([N_IN_NODE, d_comb], f16)
            nc.sync.dma_start(wct_sb[:], wct_d[:, :])
            wet_sb = spool.tile([N_IN_EDGE, N_OUT], f16)
            nc.sync.dma_start(wet_sb[:], wet_d[:, :])
            ones_sb = spool.tile([1, P], f16)
            nc.sync.dma_start(ones_sb[:], ones_d[:, :])
            brow_sb = spool.tile([1, d_comb], f16)
            nc.sync.dma_start(brow_sb[:], brow_d[:, :])
            gs_sb = spool.tile([P, E_PAD // 16], i16)
            nc.sync.dma_start(gs_sb[:], gs_d[:, :])
            gd_sb = spool.tile([P, E_PAD // 16], i16)
            nc.sync.dma_start(gd_sb[:], gd_d[:, :])

            # ---- Phase A: C pair table from xT (feature-major) ----
            with tc.tile_pool(name="a_sbuf", bufs=2) as apool, \
                 tc.tile_pool(name="a_x", bufs=1) as xpool, \
                 tc.tile_pool(name="a_ps", bufs=8, space="PSUM") as apsum:
                xt_sb = xpool.tile([P, NODES_PAD], f16)
                half = NODES_PAD // 2
                nc.sync.dma_start(xt_sb[:, :half], xt_d[:, :half])
                nc.sync.dma_start(xt_sb[:, half:], xt_d[:, half:])
                for g0 in range(0, A_TILES, GRP):
                    yc_sb = apool.tile([P, GRP, d_comb], f16, tag="yc_sb")
                    for h in range(g0, g0 + GRP, 2):
                        yc_ps = apsum.tile([P, 2 * d_comb], f32, tag="yc_ps")
                        for i in (h, h + 1):
                            nc.tensor.matmul(
                                yc_ps[:, (i - h) * d_comb:(i - h + 1) * d_comb],
                                lhsT=xt_sb[:, i * P:(i + 1) * P],
                                rhs=wct_sb[:], start=True, stop=False)
                            nc.tensor.matmul(
                                yc_ps[:, (i - h) * d_comb:(i - h + 1) * d_comb],
                                lhsT=ones_sb[:, :], rhs=brow_sb[:, :],
                                start=False, stop=True)
                        dst = yc_sb.rearrange("p g f -> p (g f)")[
                            :, (h - g0) * d_comb:(h - g0 + 2) * d_comb]
                        if (h // 2) % 2 == 0:
                            nc.scalar.activation(dst, yc_ps[:, :], Copy)
                        else:
                            nc.vector.tensor_copy(dst, yc_ps[:, :])
                    rows = c_nodes[g0 * P:(g0 + GRP) * P, :].rearrange(
                        "(g p) f -> p g f", p=P)
                    nc.sync.dma_start(rows[:, :, :], yc_sb[:, :, :])

            tc.strict_bb_all_engine_barrier()

            # ---- Phase B ----
            with tc.tile_pool(name="b_sbuf", bufs=3) as bpool, \
                 tc.tile_pool(name="b_ps", bufs=8, space="PSUM") as bpsum:
                for s in range(S_SUP):
                    grp = s // (S_SUP // N_GROUPS)
                    sp, dp = (grp >> 1) & 1, grp & 1
                    src_off = sp * 2 * N_OUT            # 0 or 512 (elems)
                    dst_off = N_OUT + dp * 2 * N_OUT    # 128 or 640... see map
                    gsrc = bpool.tile([P, K_SUP, N_OUT], f16, tag="gsrc")
                    gdst = bpool.tile([P, K_SUP, N_OUT], f16, tag="gdst")
                    for c in range(calls_per_side):
                        c0 = (s * sup_cols // 16) + c * idx_cols
                        q_base = (s * 2 * calls_per_side + 2 * c) % 4
                        nc.gpsimd.dma_gather(
                            out_ap=gsrc[:, c * 8:(c + 1) * 8, :],
                            in_ap=c_d[:, src_off:src_off + N_OUT],
                            idxs_ap=gs_sb[:, c0:c0 + idx_cols],
                            num_idxs=CALL_IDX, num_idxs_reg=CALL_IDX,
                            elem_size=N_OUT, elem_step=2 * d_comb,
                            queue_num=q_base)
                        nc.gpsimd.dma_gather(
                            out_ap=gdst[:, c * 8:(c + 1) * 8, :],
                            in_ap=c_d[:, dst_off:dst_off + N_OUT],
                            idxs_ap=gd_sb[:, c0:c0 + idx_cols],
                            num_idxs=CALL_IDX, num_idxs_reg=CALL_IDX,
                            elem_size=N_OUT, elem_step=2 * d_comb,
                            queue_num=q_base + 1)
                    eat_sb = bpool.tile([N_IN_EDGE, K_SUP, P], f16, tag="eat_sb")
                    nc.sync.dma_start(
                        eat_sb[:, :, :],
                        eat_d[:, s * sup_cols:(s + 1) * sup_cols].rearrange(
                            "f (t p) -> f t p", p=P))
                    q = bpool.tile([P, K_SUP, N_OUT], f16, tag="q")
                    nc.vector.tensor_add(q[:, :, :], gsrc[:, :, :], gdst[:, :, :])
                    outsb = bpool.tile([P, K_SUP, N_OUT], f16, tag="outsb")
                    for b in range(K_SUP // 4):
                        z_ps = bpsum.tile([P, 4 * P], f32, tag="z_ps")
                        for j in range(4):
                            nc.tensor.matmul(
                                z_ps[:, j * P:(j + 1) * P],
                                lhsT=eat_sb[:, b * 4 + j, :],
                                rhs=wet_sb[:], start=True, stop=True)
                        nc.vector.scalar_tensor_tensor(
                            outsb[:, b * 4:(b + 1) * 4, :],
                            in0=z_ps[:, :], scalar=1.0,
                            in1=q[:, b * 4:(b + 1) * 4, :],
                            op0=Alu.bypass, op1=Alu.add)
                    nc.sync.dma_start(
                        out_d[:, s * sup_cols:(s + 1) * sup_cols],
                        outsb.rearrange("p t o -> p (t o)")[:, :])

    nc.compile()
    return nc


def prep_inputs(x, edge_index, edge_attr, W, b):
    """Host-side prep: parity-group sort per core, f16 casts, layouts."""
    x = np.asarray(x, dtype=np.float32)
    edge_index = np.asarray(edge_index)
    edge_attr = np.asarray(edge_attr, dtype=np.float32)
    W = np.asarray(W, dtype=np.float32)
    b = np.asarray(b, dtype=np.float32)

    d_node = x.shape[1]
    xt = np.zeros((P, NODES_PAD), dtype=np.float16)
    xt[:, :x.shape[0]] = x.T.astype(np.float16)
    wct = np.ascontiguousarray(np.concatenate(
        [W[:, :d_node].T, W[:, d_node:2 * d_node].T], axis=1)).astype(np.float16)
    wet = np.ascontiguousarray(W[:, 2 * d_node:].T).astype(np.float16)
    ones = np.ones((1, P), dtype=np.float16)
    brow = np.zeros((1, 2 * N_OUT), dtype=np.float16)
    brow[0, :N_OUT] = b.astype(np.float16)

    src = np.ascontiguousarray(edge_index[0]).astype(np.int32)
    dst = np.ascontiguousarray(edge_index[1]).astype(np.int32)

    in_maps = []
    perms = []
    for c in range(N_CORES):
        lo, hi = c * E_CORE, (c + 1) * E_CORE
        sc, dc = src[lo:hi], dst[lo:hi]
        grp = (sc & 1) * 2 + (dc & 1)
        slot_to_edge = np.full(E_PAD, -1, dtype=np.int32)
        gs = np.zeros(E_PAD, dtype=np.int16)
        gd = np.zeros(E_PAD, dtype=np.int16)
        for g in range(N_GROUPS):
            idx_e = np.nonzero(grp == g)[0]
            n = idx_e.size
            assert n <= GROUP_SLOTS, f"parity group overflow: {n}"
            base = g * GROUP_SLOTS
            slot_to_edge[base:base + n] = idx_e
            gs[base:base + n] = (sc[idx_e] >> 1).astype(np.int16)
            gd[base:base + n] = (dc[idx_e] >> 1).astype(np.int16)
        valid = slot_to_edge >= 0
        ea_slot = np.zeros((E_PAD, N_IN_EDGE), dtype=np.float16)
        ea_slot[valid] = edge_attr[lo + slot_to_edge[valid]].astype(np.float16)
        eat = np.ascontiguousarray(ea_slot.T)
        in_maps.append({
            "xt": xt, "wct": wct, "wet": wet, "ones": ones, "brow": brow,
            "gs": _idx_wrap16(gs, CALL_IDX), "gd": _idx_wrap16(gd, CALL_IDX),
            "eat": eat,
        })
        perms.append(slot_to_edge)
    return in_maps, perms


def _idx_wrap16(seq_i16, n_idx):
    """Pack a flat int16 index sequence into the dma_gather SBUF layout:
    index i of each n_idx-call at (partition i%16, column i//16), replicated
    to 8x16 partition rows."""
    cols = n_idx // 16
    blocks = seq_i16.reshape(-1, cols, 16)
    arr = blocks.transpose(0, 2, 1).reshape(-1, 16, cols)
    out = np.concatenate([np.tile(a, (8, 1)) for a in arr], axis=1)
    return np.ascontiguousarray(out)


def unpack_outputs(res, perms):
    outs = []
    for c in range(N_CORES):
        o = res.results[c]["out"]                       # [128, E_PAD] f16
        # cols are (s, t, chan); slot = s*2048 + t*128 + p
        rows = np.ascontiguousarray(
            o.reshape(P, S_SUP, K_SUP, N_OUT).transpose(1, 2, 0, 3)
            .reshape(E_PAD, N_OUT))
        slot_to_edge = perms[c]
        valid = slot_to_edge >= 0
        oc = np.empty((E_CORE, N_OUT), dtype=np.float32)
        oc[slot_to_edge[valid]] = rows[valid].astype(np.float32)
        outs.append(oc)
    return np.concatenate(outs, axis=0)


_NC_CACHE = {}


def _get_program():
    key = "full"
    if key not in _NC_CACHE:
        _NC_CACHE[key] = build_program()
    return _NC_CACHE[key]


def run_on_hw(in_maps, nc=None, trace=False):
    from concourse import bass_utils
    if nc is None:
        nc = _get_program()
    kw = {}
    if trace:
        _install_profile_hook(bass_utils)
        kw["trace"] = True
    res = bass_utils.run_bass_kernel_spmd(
        nc, in_maps, core_ids=list(range(N_CORES)), **kw)
    return res


def _install_profile_hook(bass_utils):
    """Inject the NTFF profile hook missing from this image's antenv."""
    import types
    if "antenv.axon_hooks" in sys.modules:
        return
    try:
        from trn_agent_boot.trn_boot import _ntff_profile_via_ctypes
        hook = _ntff_profile_via_ctypes("/opt/axon/libaxon_pjrt.so")
    except Exception:
        hook = None
    mod = types.ModuleType("antenv.axon_hooks")
    mod.get_axon_ntff_profile_hook = lambda: hook
    mod.set_axon_ntff_profile_hook = lambda h: None
    sys.modules["antenv.axon_hooks"] = mod
    bass_utils.upload_artifacts = lambda tmpdir: f"file://{tmpdir}"


def kernel(x, edge_index, edge_attr, W, b):
    in_maps, perms = prep_inputs(x, edge_index, edge_attr, W, b)
    res = run_on_hw(in_maps)
    return unpack_outputs(res, perms)
